# revision 30
# baseline (speedup 1.0000x reference)
"""Trainium2 Bass kernel for an AttentionBlock (GroupNorm + QKV + MHA + proj
+ residual), data-parallel over the batch across 8 NeuronCores.

Contract: kernel(**inputs) takes the FULL inputs of reference.setup_inputs()
and returns the FULL [8, 512, 32, 32] float32 output.

Per-core layout (core i handles batch element i, x viewed as [C=512, L=1024]):
  - GroupNorm(32 groups) via bn_stats per channel + tiny group-reduce matmuls.
  - QKV matmul in float32r with output rows permuted into pair-chunks
    [q0;q1]..[q6;q7], [k0;k1].., [v0;v1].. so that the two heads of a pair
    occupy partitions 0:64 / 64:128 of one 128-partition chunk.
  - scoresT[s, t] = k^T q per head, two heads packed in the PE array
    concurrently (K=64 row tiling via base partitions 0 / 64).
  - softmax without max-subtraction (logits are O(1) for this model):
    exp on the Scalar engine straight out of PSUM (fp32 -> bf16, FD=2048).
  - AV matmul with stationary operand [vT | ones] so rows 64:128 of the
    output accumulate sum(exp) per (head, t); normalization is a DVE divide.
  - proj matmul in float32r + fused (bias + residual) via scalar_tensor_tensor.
"""

import contextlib

import numpy as np

try:
    import jax as _jax
    _jax.config.update("jax_compilation_cache_dir", "/tmp/jax_neff_cache")
    _jax.config.update("jax_persistent_cache_min_compile_time_secs", 0.0)
except Exception:
    pass

import concourse.bass as bass
import concourse.tile as tile
from concourse import mybir
from concourse.bass_utils import run_bass_kernel_spmd

F32 = mybir.dt.float32
F32R = mybir.dt.float32r
BF16 = mybir.dt.bfloat16
FT = mybir.ActivationFunctionType
ALU = mybir.AluOpType

B, C, HH, WW = 8, 512, 32, 32
L = HH * WW            # 1024
NH = 8                 # heads
CH = C // NH           # 64 channels per head
NG = 32                # groupnorm groups
GS = C // NG           # 16 channels per group
EPS = 1e-5
NCHUNK = C // 128      # 4 partition chunks of channels
N_CORES = 8


def _split_excess_waits(nc, default_max=1, ctrl_max=1):
    """walrus only encodes 1 sync wait on CTRL-like instructions (Drain/NoOp)
    and 2 on regular ones; split extra waits onto preceding NoOp carriers."""
    n_split = 0
    for f in nc.m.functions:
        for bb in f.blocks:
            insts = bb.instructions
            i = 0
            while i < len(insts):
                inst = insts[i]
                si = inst.sync_info
                cap = (
                    ctrl_max
                    if isinstance(inst, (mybir.InstDrain, mybir.InstNoOp))
                    else default_max
                )
                if si is not None and si.on_wait and len(si.on_wait) > cap:
                    waits = list(si.on_wait)
                    keep, extra = waits[-cap:], waits[:-cap]
                    carriers = [
                        mybir.InstNoOp(
                            name=f"{inst.name}-wsplit-{j}",
                            engine=inst.engine,
                            sync_info=mybir.SyncInfo(
                                on_wait=[w], on_update=[]
                            ),
                            bass_nofuse=True,
                        )
                        for j, w in enumerate(extra)
                    ]
                    inst.sync_info = mybir.SyncInfo(
                        on_wait=keep, on_update=list(si.on_update or [])
                    )
                    for k, c in enumerate(carriers):
                        insts.insert(i + k, c)
                    i += len(carriers)
                    n_split += 1
                i += 1
    return n_split


def build_nc(split_waits=True):
    nc = bass.Bass("TRN2", debug=False)

    x_d = nc.dram_tensor("x", [C, L], F32, kind="ExternalInput")
    qkvt_d = nc.dram_tensor("qkvt", [C, 3 * C], F32R, kind="ExternalInput")
    qkb_d = nc.dram_tensor("qkb", [128, 8], F32, kind="ExternalInput")
    projt_d = nc.dram_tensor("projt", [C, C], F32R, kind="ExternalInput")
    projb_d = nc.dram_tensor("projb", [128, NCHUNK], F32, kind="ExternalInput")
    gnw_d = nc.dram_tensor("gnw", [128, NCHUNK], F32, kind="ExternalInput")
    gnb_d = nc.dram_tensor("gnb", [128, NCHUNK], F32, kind="ExternalInput")
    gnind_d = nc.dram_tensor("gnind", [128, NCHUNK * NG], F32, kind="ExternalInput")
    gnexp_d = nc.dram_tensor("gnexp", [NG, NCHUNK * 128], F32, kind="ExternalInput")
    out_d = nc.dram_tensor("out", [C, L], F32, kind="ExternalOutput")
    ses_d = nc.dram_tensor("sesdram", [NCHUNK, 2, 2048], F32)

    with tile.TileContext(nc) as tc, contextlib.ExitStack() as top:
        consts = top.enter_context(tc.tile_pool(name="consts", bufs=1))
        xpool = top.enter_context(tc.tile_pool(name="x", bufs=1))
        qkpool = top.enter_context(tc.tile_pool(name="qk", bufs=1))
        vtpool = top.enter_context(tc.tile_pool(name="vt", bufs=1))
        apool = top.enter_context(tc.tile_pool(name="a", bufs=1))
        pipe = contextlib.ExitStack()
        wqpool = pipe.enter_context(tc.tile_pool(name="wq", bufs=1))

        # ---- input loads (x first: GroupNorm is the critical path) ----------
        xs = []
        for c in range(NCHUNK):
            t = xpool.tile([128, L], F32, tag=f"x{c}")
            nc.sync.dma_start(out=t, in_=x_d.ap()[c * 128:(c + 1) * 128, :])
            xs.append(t)
        gnw = consts.tile([128, NCHUNK], F32)
        nc.sync.dma_start(out=gnw, in_=gnw_d.ap())
        gnb = consts.tile([128, NCHUNK], F32)
        nc.sync.dma_start(out=gnb, in_=gnb_d.ap())
        gnind = consts.tile([128, NCHUNK * NG], F32)
        nc.sync.dma_start(out=gnind, in_=gnind_d.ap())
        gnexp = consts.tile([NG, NCHUNK * 128], F32)
        nc.sync.dma_start(out=gnexp, in_=gnexp_d.ap())
        epsv = consts.tile([NG, 1], F32)
        nc.vector.memset(epsv, EPS)
        # prefetch the Sqrt activation table while DMAs run
        sqrt_warm = consts.tile([NG, 1], F32)
        nc.scalar.activation(out=sqrt_warm, in_=epsv, func=FT.Sqrt)

        wq = []
        for c in range(NCHUNK):
            t = wqpool.tile([128, 3 * C], F32R, tag=f"wq{c}")
            nc.sync.dma_start(out=t, in_=qkvt_d.ap()[c * 128:(c + 1) * 128, :])
            wq.append(t)
        pw = []
        for c in range(NCHUNK):
            t = consts.tile([128, C], F32R, tag=f"pw{c}")
            nc.sync.dma_start(out=t, in_=projt_d.ap()[c * 128:(c + 1) * 128, :])
            pw.append(t)
        qkb = consts.tile([128, 8], F32)
        nc.sync.dma_start(out=qkb, in_=qkb_d.ap())
        projb = consts.tile([128, NCHUNK], F32)
        nc.sync.dma_start(out=projb, in_=projb_d.ap())

        # ---- GroupNorm ------------------------------------------------------
        with contextlib.ExitStack() as gctx:
            gsb = gctx.enter_context(tc.tile_pool(name="gn_sb", bufs=1))
            gps = gctx.enter_context(tc.tile_pool(name="gn_ps", bufs=2, space="PSUM"))

            stats3 = gsb.tile([128, NCHUNK, 3], F32)
            for c in range(NCHUNK):
                st6 = gsb.tile([128, 2, 6], F32, tag="st6")
                nc.vector.bn_stats(out=st6[:, 0, :], in_=xs[c][:, 0:512])
                nc.vector.bn_stats(out=st6[:, 1, :], in_=xs[c][:, 512:1024])
                mv = gsb.tile([128, 2], F32, tag="mv")
                nc.vector.bn_aggr(out=mv, in_=st6)
                nc.vector.tensor_copy(stats3[:, c, 0:2], mv)
                nc.vector.tensor_tensor(
                    out=stats3[:, c, 2:3], in0=mv[:, 0:1], in1=mv[:, 0:1],
                    op=ALU.mult,
                )
            gst = gps.tile([NG, 3], F32)
            for c in range(NCHUNK):
                nc.tensor.matmul(
                    gst,
                    lhsT=gnind[:, c * NG:(c + 1) * NG],
                    rhs=stats3[:, c, :],
                    start=(c == 0), stop=(c == NCHUNK - 1),
                )
            # group stats: [gmean, mean_of_var, mean_of_mean2]
            grs = gsb.tile([NG, 3], F32)
            nc.vector.tensor_copy(grs, gst)
            gvar = gsb.tile([NG, 1], F32)
            nc.vector.tensor_tensor(out=gvar, in0=grs[:, 1:2], in1=grs[:, 2:3], op=ALU.add)
            m2 = gsb.tile([NG, 1], F32)
            nc.vector.tensor_tensor(out=m2, in0=grs[:, 0:1], in1=grs[:, 0:1], op=ALU.mult)
            nc.vector.tensor_tensor(out=gvar, in0=gvar, in1=m2, op=ALU.subtract)
            # grs2: col0 = gmean, col1 = rstd
            grs2 = gsb.tile([NG, 2], F32)
            nc.vector.tensor_copy(grs2[:, 0:1], grs[:, 0:1])
            sd = gsb.tile([NG, 1], F32)
            nc.scalar.activation(out=sd, in_=gvar, func=FT.Sqrt, bias=epsv, scale=1.0)
            nc.vector.reciprocal(out=grs2[:, 1:2], in_=sd)

            alpha = gsb.tile([128, NCHUNK], F32)
            beta = gsb.tile([128, NCHUNK], F32)
            for c in range(NCHUNK):
                cs = gps.tile([128, 2], F32, tag="cs")
                nc.tensor.matmul(
                    cs,
                    lhsT=gnexp[:, c * 128:(c + 1) * 128],
                    rhs=grs2,
                    start=True, stop=True,
                )
                nc.vector.tensor_tensor(
                    out=alpha[:, c:c + 1], in0=cs[:, 1:2], in1=gnw[:, c:c + 1],
                    op=ALU.mult,
                )
                ngm = gsb.tile([128, 1], F32, tag="ngm")
                nc.vector.tensor_scalar(
                    out=ngm, in0=cs[:, 0:1], scalar1=-1.0, scalar2=None, op0=ALU.mult,
                )
                nc.vector.scalar_tensor_tensor(
                    out=beta[:, c:c + 1], in0=ngm, scalar=alpha[:, c:c + 1],
                    in1=gnb[:, c:c + 1], op0=ALU.mult, op1=ALU.add,
                )
            # xn = x * alpha + beta  (float32r so it can feed f32r matmuls)
            xn = []
            for c in range(NCHUNK):
                t = wqpool.tile([128, L], F32R, tag=f"xn{c}")
                nc.scalar.activation(
                    out=t, in_=xs[c], func=FT.Identity,
                    scale=alpha[:, c:c + 1], bias=beta[:, c:c + 1],
                )
                xn.append(t)
            # prefetch the Exp activation table (overlaps the QKV matmuls)
            exp_warm = gsb.tile([NG, 1], F32)
            nc.scalar.activation(out=exp_warm, in_=sd, func=FT.Exp)

        # ---- fused QKV + attention software pipeline ------------------------
        # PE in-order stream interleaves, per attention step of pair p:
        #   QK mms (pair p) | AV mms (pair p-1, half-split) | QKV mms (pair p+1)
        # so the Scalar engine's exp stream paces the whole middle section.
        # q/k: [o-chunk, l] outputs drained with bias -> bf16.
        # v: computed TRANSPOSED (xn stationary, v-weight cols moving) so the
        #    AV stationary operand [vT | ones] needs no separate transpose.
        qp, kp = [None] * 4, [None] * 4
        vth = []
        for h in range(NH):
            t = vtpool.tile([128, 8, 128], BF16, tag=f"vt{h}")
            nc.gpsimd.memset(t[:, :, 64:128], 1.0)
            vth.append(t)
        ach = []
        avps = pipe.enter_context(tc.tile_pool(name="av_ps", bufs=1, space="PSUM"))
        qps = pipe.enter_context(tc.tile_pool(name="qkv_ps", bufs=2, space="PSUM"))
        wtp_pool = pipe.enter_context(tc.tile_pool(name="wt", bufs=2))
        aupool = pipe.enter_context(tc.tile_pool(name="aun", bufs=2))
        rpool = pipe.enter_context(tc.tile_pool(name="rcp", bufs=2))
        sps_cm = tc.tile_pool(name="sc_ps", bufs=2, space="PSUM")
        sps = sps_cm.__enter__()

        def qkv_units(p):
            """Per-pair QKV work as a list of ~4-matmul emission units."""
            units = []
            for m in (p, 4 + p):
                tname = f"{'q' if m < 4 else 'k'}{m % 2}"
                t = qkpool.tile([128, L], BF16, tag=tname, name=f"qk{m}")
                (qp if m < 4 else kp)[m % 4] = t

                def qk_half(m=m, t=t, half=0):
                    pt = qps.tile([128, 512], F32, tag="qkvps", name=f"qkp{m}_{half}")
                    for c in range(NCHUNK):
                        nc.tensor.matmul(
                            pt,
                            lhsT=wq[c][:, m * 128:(m + 1) * 128],
                            rhs=xn[c][:, half * 512:(half + 1) * 512],
                            start=(c == 0), stop=(c == NCHUNK - 1),
                        )
                    dst = t[:, half * 512:(half + 1) * 512]
                    if m >= 4:
                        # k: additive bias cancels in softmax over s - skip it
                        if m % 4 <= 1:
                            nc.scalar.copy(out=dst, in_=pt)
                        else:
                            nc.vector.tensor_copy(dst, pt)
                    elif m <= 1:
                        nc.scalar.activation(
                            out=dst, in_=pt, func=FT.Identity,
                            bias=qkb[:, m:m + 1],
                        )
                    else:
                        nc.vector.tensor_scalar(
                            out=dst, in0=pt,
                            scalar1=qkb[:, m:m + 1], scalar2=None, op0=ALU.add,
                        )
                units.append(lambda m=m, t=t: qk_half(m, t, 0))
                units.append(lambda m=m, t=t: qk_half(m, t, 1))
            vcols = slice(1024 + p * 128, 1024 + (p + 1) * 128)
            for g in range(2):
                vt_ps_box = {}

                def v_block(g=g, b=0, box=None):
                    if b == 0:
                        box["t"] = qps.tile(
                            [128, 4, 128], F32, tag="qkvps", name=f"vtp{p}_{g}",
                        )
                    vt_ps = box["t"]
                    i = g * 4 + b
                    for c in range(NCHUNK):
                        nc.tensor.matmul(
                            vt_ps[:, b, :],
                            lhsT=xn[c][:, i * 128:(i + 1) * 128],
                            rhs=wq[c][:, vcols],
                            start=(c == 0), stop=(c == NCHUNK - 1),
                        )
                    if b == 3:
                        nc.vector.tensor_copy(
                            vth[2 * p][:, g * 4:(g + 1) * 4, 0:64],
                            vt_ps[:, :, 0:64],
                        )
                        nc.vector.tensor_copy(
                            vth[2 * p + 1][:, g * 4:(g + 1) * 4, 0:64],
                            vt_ps[:, :, 64:128],
                        )
                for b in range(4):
                    units.append(lambda g=g, b=b, box=vt_ps_box: v_block(g, b, box))
            return units

        def qk_exp_step(p, i, wt):
            sta = sps.tile([128, L], F32, tag="sc", name=f"sca{p}_{i}")
            stb = sps.tile([128, L], F32, tag="sc", name=f"scb{p}_{i}")
            for n in range(2):
                for hb, st in ((0, sta), (64, stb)):
                    nc.tensor.matmul(
                        st[:, n * 512:(n + 1) * 512],
                        lhsT=kp[p][hb:hb + 64, i * 128:(i + 1) * 128],
                        rhs=qp[p][hb:hb + 64, n * 512:(n + 1) * 512],
                        start=True, stop=True,
                        tile_position=(hb, 0),
                    )
            nc.scalar.activation(out=wt[:, i, 0:1024], in_=sta, func=FT.Exp)
            nc.scalar.activation(out=wt[:, i, 1024:2048], in_=stb, func=FT.Exp)

        def av_half_step(p, half, j, wt, av):
            """Accumulate wt slice j into av (one t-half for both heads)."""
            for hi, hoff in ((2 * p, 0), (2 * p + 1, 512)):
                nc.tensor.matmul(
                    av[:, hoff:hoff + 512],
                    lhsT=vth[hi][:, j, :],
                    rhs=wt[:, j, (hi % 2) * 1024 + half * 512:
                              (hi % 2) * 1024 + half * 512 + 512],
                    start=(j == 0), stop=(j == 7),
                )

        def drain_av(p, half, av, aun):
            nc.vector.tensor_copy(aun[:, half * 1024:(half + 1) * 1024], av[0:65, :])

        a_ts = {}

        def norm_half(p, half, aun):
            # aun cols: [a t-lo | b t-lo | a t-hi | b t-hi], row 64 = sumexp
            cs = slice(half * 1024, (half + 1) * 1024)
            nc.sync.dma_start(out=ses_d.ap()[p, 0, cs], in_=aun[64:65, cs])
            sesw = rpool.tile([128, 8], F32, tag="sesw", name=f"sesw{p}_{half}")
            nc.sync.dma_start(
                out=sesw,
                in_=ses_d.ap()[p, 0, cs].rearrange("(p f) -> p f", p=128),
            )
            nc.vector.reciprocal(out=sesw, in_=sesw)
            nc.sync.dma_start(out=ses_d.ap()[p, 1, cs], in_=sesw)
            rb = rpool.tile([64, 1024], F32, tag="rb", name=f"rb{p}_{half}")
            row = ses_d.ap()[p, 1, cs]
            rb_src = bass.AP(
                tensor=row.tensor, offset=row.offset,
                ap=[[0, 64]] + list(row.ap),
            )
            nc.sync.dma_start(out=rb, in_=rb_src)
            if half == 0:
                a_ts[p] = apool.tile([128, L], F32R, tag=f"a{p}", name=f"a{p}")
            a_t = a_ts[p]
            for hh, base in ((0, 0), (64, 512)):
                nc.vector.tensor_tensor(
                    out=a_t[hh:hh + 64, half * 512:(half + 1) * 512],
                    in0=aun[0:64, half * 1024 + base: half * 1024 + base + 512],
                    in1=rb[0:64, base:base + 512],
                    op=ALU.mult,
                )
            if half == 1:
                ach.append(a_t)

        # ---- pipeline schedule ----------------------------------------------
        for u in qkv_units(0):
            u()
        pending_qkv = qkv_units(1)
        wts, avs, auns = {}, {}, {}
        wts[0] = wtp_pool.tile([128, 8, 2048], BF16, tag="wt", name="wt0")
        for p in range(NCHUNK):
            if p + 1 < NCHUNK and p >= 1:
                pending_qkv = qkv_units(p + 1)
            for i in range(8):
                qk_exp_step(p, i, wts[p])
                if p >= 1:
                    pm = p - 1
                    SCHED = {0: (0, [0, 1, 2]), 1: (0, [3, 4, 5]),
                             2: (0, [6, 7]), 3: (1, [0, 1, 2]),
                             4: (1, [3, 4, 5]), 5: (1, [6, 7])}
                    if i == 0:
                        avs[pm] = avps.tile(
                            [128, 1024], F32, tag="av", name=f"avlo{pm}")
                        auns[pm] = aupool.tile(
                            [65, 2048], F32, tag="aun", name=f"aun{pm}")
                    if i == 3:
                        drain_av(pm, 0, avs[pm], auns[pm])
                        norm_half(pm, 0, auns[pm])
                        avs[pm] = avps.tile(
                            [128, 1024], F32, tag="av", name=f"avhi{pm}")
                    if i in SCHED:
                        half, js = SCHED[i]
                        for j in js:
                            av_half_step(pm, half, j, wts[pm], avs[pm])
                    if i == 6:
                        drain_av(pm, 1, avs[pm], auns[pm])
                    if i == 7:
                        norm_half(pm, 1, auns[pm])
                # pair 3: run its lo-half AV in the idle qkv psum slots
                if p == NCHUNK - 1:
                    if i == 0:
                        auns[p] = aupool.tile(
                            [65, 2048], F32, tag="aun", name=f"aun{p}")
                        av3 = {}
                        av3["a"] = qps.tile(
                            [128, 512], F32, tag="qkvps", name="av3lo_a")
                        av3["b"] = qps.tile(
                            [128, 512], F32, tag="qkvps", name="av3lo_b")
                        avs["lo3"] = av3
                    av3 = avs["lo3"]
                    for hk, hi_, woff in (("a", 2 * p, 0), ("b", 2 * p + 1, 1024)):
                        nc.tensor.matmul(
                            av3[hk],
                            lhsT=vth[hi_][:, i, :],
                            rhs=wts[p][:, i, woff:woff + 512],
                            start=(i == 0), stop=(i == 7),
                        )
                # spread next pair's QKV units across this pair's steps
                lo = (len(pending_qkv) * i) // 8
                hi = (len(pending_qkv) * (i + 1)) // 8
                for u in pending_qkv[lo:hi]:
                    u()
            if p + 1 < NCHUNK:
                wts[p + 1] = wtp_pool.tile(
                    [128, 8, 2048], BF16, tag="wt", name=f"wt{p+1}")
            pending_qkv = []
        # last exps done: release the scores banks so proj partials can start
        sps_cm.__exit__(None, None, None)
        pps_cm = tc.tile_pool(name="pr_ps", bufs=2, space="PSUM")
        pps = pps_cm.__enter__()
        o_cm = tc.tile_pool(name="o", bufs=2)
        opool = o_cm.__enter__()

        def proj_mms(pt, m, cs):
            for c in cs:
                for n in range(2):
                    nc.tensor.matmul(
                        pt[:, n * 512:(n + 1) * 512],
                        lhsT=pw[c][:, m * 128:(m + 1) * 128],
                        rhs=ach[c][:, n * 512:(n + 1) * 512],
                        start=(c == 0), stop=(c == NCHUNK - 1),
                    )

        def proj_finish(pt, m):
            ot = opool.tile([128, L], F32, tag="ot", name=f"ot{m}")
            nc.vector.scalar_tensor_tensor(
                out=ot, in0=pt, scalar=projb[:, m:m + 1], in1=xs[m],
                op0=ALU.add, op1=ALU.add,
            )
            nc.sync.dma_start(
                out=out_d.ap()[m * 128:(m + 1) * 128, :], in_=ot,
            )

        # proj partials for m=0,1 over ach[0:3] run under the last pair's AV
        prt = {}
        for m in range(2):
            prt[m] = pps.tile([128, L], F32, tag="prps", name=f"pr{m}")
            proj_mms(prt[m], m, range(NCHUNK - 1))

        # epilogue: finish the last pair (lo already accumulated in qps slots)
        pm = NCHUNK - 1
        av3 = avs["lo3"]
        nc.vector.tensor_copy(auns[pm][:, 0:512], av3["a"][0:65, :])
        nc.vector.tensor_copy(auns[pm][:, 512:1024], av3["b"][0:65, :])
        norm_half(pm, 0, auns[pm])
        av = avps.tile([128, 1024], F32, tag="av", name=f"av{pm}_hi")
        for j in range(8):
            av_half_step(pm, 1, j, wts[pm], av)
        drain_av(pm, 1, av, auns[pm])
        norm_half(pm, 1, auns[pm])

        for m in range(2):
            proj_mms(prt[m], m, [NCHUNK - 1])
            proj_finish(prt[m], m)
        for m in range(2, NCHUNK):
            pt = pps.tile([128, L], F32, tag="prps", name=f"pr{m}")
            proj_mms(pt, m, range(NCHUNK))
            proj_finish(pt, m)

        o_cm.__exit__(None, None, None)
        pps_cm.__exit__(None, None, None)
        pipe.close()

    if split_waits:
        _split_excess_waits(nc)
    return nc


def prep_inputs(x, gn_w, gn_b, qkv_w, qkv_b, proj_w, proj_b):
    """Host-side prep: permute/scale QKV weights, fold biases, GN indicators."""
    x = np.ascontiguousarray(np.asarray(x, dtype=np.float32)).reshape(B, C, L)
    qkv_w = np.asarray(qkv_w, dtype=np.float32)
    qkv_b = np.asarray(qkv_b, dtype=np.float32)
    proj_w = np.asarray(proj_w, dtype=np.float32)
    proj_b = np.asarray(proj_b, dtype=np.float32)
    gn_w = np.asarray(gn_w, dtype=np.float32)
    gn_b = np.asarray(gn_b, dtype=np.float32)

    # output-row permutation: q pair-chunks, k pair-chunks, v pair-chunks
    perm = np.empty(3 * C, dtype=np.int64)
    pos = 0
    for part in range(3):             # 0=q, 1=k, 2=v
        for h in range(NH):
            rows = h * 3 * CH + part * CH + np.arange(CH)
            perm[pos:pos + CH] = rows
            pos += CH
    w_perm = qkv_w[perm, :].copy()
    b_perm = qkv_b[perm].copy()
    w_perm[0:C] *= 0.125              # fold softmax scale^2 into q
    b_perm[0:C] *= 0.125

    qkvt = np.ascontiguousarray(w_perm.T)                      # [C, 3C]
    qkb = np.ascontiguousarray(b_perm[0:2 * C].reshape(8, 128).T)  # [128, 8]
    bv = b_perm[2 * C:3 * C]                                   # v bias, head-major == channel order
    projt = np.ascontiguousarray(proj_w.T)                     # [C, C]
    projb = np.ascontiguousarray(
        (proj_b + proj_w @ bv).reshape(NCHUNK, 128).T)         # [128, 4]
    gnw_t = np.ascontiguousarray(gn_w.reshape(NCHUNK, 128).T)  # [128, 4]
    gnb_t = np.ascontiguousarray(gn_b.reshape(NCHUNK, 128).T)

    gnind = np.zeros((128, NCHUNK * NG), np.float32)
    gnexp = np.zeros((NG, NCHUNK * 128), np.float32)
    for c in range(NCHUNK):
        for p in range(128):
            g = (c * 128 + p) // GS
            gnind[p, c * NG + g] = 1.0 / GS
            gnexp[g, c * 128 + p] = 1.0
    shared = {
        "qkvt": qkvt, "qkb": qkb, "projt": projt, "projb": projb,
        "gnw": gnw_t, "gnb": gnb_t, "gnind": gnind, "gnexp": gnexp,
    }
    in_maps = [
        {"x": np.ascontiguousarray(x[i]), **shared} for i in range(N_CORES)
    ]
    return in_maps


_NC_CACHE = {}


def _get_nc():
    if "nc" not in _NC_CACHE:
        _NC_CACHE["nc"] = build_nc()
    return _NC_CACHE["nc"]


def kernel(x, gn_w, gn_b, qkv_w, qkv_b, proj_w, proj_b, _trace=False, _tmpdir=None):
    nc = _get_nc()
    in_maps = prep_inputs(x, gn_w, gn_b, qkv_w, qkv_b, proj_w, proj_b)
    res = run_bass_kernel_spmd(
        nc, in_maps, core_ids=list(range(N_CORES)), trace=_trace, tmpdir=_tmpdir,
    )
    out = np.stack([res.results[i]["out"] for i in range(N_CORES)], axis=0)
    out = out.reshape(B, C, HH, WW).astype(np.float32)
    if _trace:
        kernel.last_results = res
    return out
